# revision 8
# baseline (speedup 1.0000x reference)
"""Differential attention kernel for 8 Trainium2 NeuronCores.

Sharding: batch x head-group. Core c handles batch b = c//4, heads
[4g, 4g+4) with g = c%4. Each core computes Q/K/V projections for its
heads over the full sequence, causal differential attention, and its
partial O-projection; the host sums the 4 partials per batch.

Differential attention trick: score = (q1.k1 - lam*q2.k2) * scale is a
single K=128 matmul with stacked [q1*scale; -lam*scale*q2] and [k1; k2]
head vectors (scales folded into the projection weights on the host).

Structure: a software pipeline over 4 query windows of 512. The
prologue projects qT/kT/V for window 0; each iteration nj runs
attention for window nj (ACT-bound: exp) while the tensor engine fills
idle cycles with the projections for window nj+1 and the O-projection
+ output DMA of window nj. PSUM budget: scores 2x[128,1024] (4 banks)
+ attention accumulators 2x[65,512] pair-serial (2 banks) + a shared
chain pool for all projection chains (2 banks).

Softmax: scores are computed transposed (keys on partitions, queries
free), exp'd without max subtraction (inputs are bounded; exp is exact
to 2ULP on ACT), and the denominator comes for free from a ones-column
appended to V in the P@V matmul. Causality is applied structurally
(upper blocks skipped, diagonal blocks trimmed to N=512-128*uu and
zeroed post-exp), which the host validates against the attention_mask
input before dispatch.
"""
import math
from contextlib import ExitStack

import numpy as np
import ml_dtypes

S = 2048
H = 2048
NH = 16
HD = 64
NHC = 4          # heads per core
BF = ml_dtypes.bfloat16

_CACHED_NC = None


def _build_nc():
    import concourse.mybir as mybir
    import concourse.tile as tile
    from concourse import bacc

    bf16 = mybir.dt.bfloat16
    f32 = mybir.dt.float32
    AF = mybir.ActivationFunctionType

    nc = bacc.Bacc(None, target_bir_lowering=False)
    hT = nc.declare_dram_parameter("hT", [H, S], bf16, isOutput=False)
    wq = nc.declare_dram_parameter("wq", [H, NHC * 128], bf16, isOutput=False)
    wk = nc.declare_dram_parameter("wk", [H, NHC * 128], bf16, isOutput=False)
    wv = nc.declare_dram_parameter("wv", [H, NHC * 65], bf16, isOutput=False)
    wo = nc.declare_dram_parameter("wo", [128, 2, S], bf16, isOutput=False)
    out = nc.declare_dram_parameter("out", [S, H], bf16, isOutput=True)

    KT = H // 128    # 16 contraction tiles for projections
    NQ = S // 512    # 4 query windows
    NS = S // 128    # 16 seq tiles

    with tile.TileContext(nc) as tc:
        with ExitStack() as ctx:
            # ---- persistent SBUF ----
            sb = ctx.enter_context(tc.tile_pool(name="sb", bufs=1))
            qk_sb = ctx.enter_context(tc.tile_pool(name="qk", bufs=1))
            ht_all = sb.tile([128, KT, S], bf16)          # hidden^T
            wq_sb = sb.tile([128, KT, NHC * 128], bf16)
            wk_sb = sb.tile([128, KT, NHC * 128], bf16)
            wv_sb = sb.tile([128, KT, NHC * 65], bf16)
            wo_sb = sb.tile([128, 2, S], bf16)            # head-pair stacked Wo rows
            qT = qk_sb.tile([128, NHC, S], bf16)          # [q1*s; -lam*s*q2] per head
            kT = qk_sb.tile([128, NHC, S], bf16)          # [k1; k2] per head
            v4 = qk_sb.tile([128, NS, NHC * 65], bf16)    # V tiles + ones cols
            avt = qk_sb.tile([128, 2, S], bf16)           # attn_out^T, head pairs stacked
            # warm the ACT exp table while DMAs stream in
            warm = sb.tile([1, 16], f32)
            nc.vector.memset(warm[:], 0.0)
            nc.scalar.activation(warm[:], warm[:], AF.Exp)

            # ---- input DMAs: k-ordered contiguous rows so projection
            # chains can start as soon as the first rows land ----
            for k in range(KT):
                nc.sync.dma_start(out=ht_all[:, k, :], in_=hT[k * 128:(k + 1) * 128, :])
                nc.sync.dma_start(out=wq_sb[:, k, :], in_=wq[k * 128:(k + 1) * 128, :])
                nc.sync.dma_start(out=wk_sb[:, k, :], in_=wk[k * 128:(k + 1) * 128, :])
                nc.sync.dma_start(out=wv_sb[:, k, :], in_=wv[k * 128:(k + 1) * 128, :])
            nc.sync.dma_start(out=wo_sb[:], in_=wo[:, :, :])

            # shared PSUM chain pool: qk/v/o projection chains (2 banks)
            chp = ctx.enter_context(tc.tile_pool(name="chp", bufs=2, space="PSUM"))
            att_work = ctx.enter_context(tc.tile_pool(name="attw", bufs=3))
            nrm_work = ctx.enter_context(tc.tile_pool(name="nrmw", bufs=2))
            oout_sb = ctx.enter_context(tc.tile_pool(name="oout", bufs=3))
            atp = ctx.enter_context(tc.tile_pool(name="atp", bufs=2, space="PSUM"))

            def qk_window(w):
                ws = slice(w * 512, (w + 1) * 512)
                for h in range(NHC):
                    hs = slice(h * 128, (h + 1) * 128)
                    for w_sb, dst in ((wq_sb, qT), (wk_sb, kT)):
                        pp = chp.tile([128, 512], f32, tag="chain")
                        for k in range(KT):
                            nc.tensor.matmul(pp[:],
                                             lhsT=w_sb[:, k, hs],
                                             rhs=ht_all[:, k, ws],
                                             start=(k == 0), stop=(k == KT - 1))
                        nc.vector.tensor_copy(dst[:, h, ws], pp[:])

            def v_window(w):
                for st in range(4 * w, 4 * w + 4):
                    vp = chp.tile([128, 512], f32, tag="chain")
                    for k in range(KT):
                        nc.tensor.matmul(vp[:, 0:NHC * 65],
                                         lhsT=ht_all[:, k, st * 128:(st + 1) * 128],
                                         rhs=wv_sb[:, k, :],
                                         start=(k == 0), stop=(k == KT - 1))
                    nc.vector.tensor_copy(v4[:, st, :], vp[:, 0:NHC * 65])
                    for j in range(NHC):
                        nc.gpsimd.memset(v4[:, st, j * 65 + 64:j * 65 + 65], 1.0)

            def attention(nj):
                qs = slice(nj * 512, (nj + 1) * 512)
                nblk = 4 * nj + 4
                for hp in range(2):
                    pair_heads = (2 * hp, 2 * hp + 1)
                    pav = {}
                    for h in pair_heads:
                        pav[h] = atp.tile([65, 512], f32, tag="av", bufs=2,
                                          name=f"av{h % 2}")
                    # round-robin the two heads per key-block group so one
                    # head's score matmuls hide the other head's exp
                    for kg in range(nblk // 2):
                        scs, pts = {}, {}
                        g0 = 256 if kg == 2 * nj + 1 else 0
                        for h in pair_heads:
                            sc = atp.tile([128, 1024], f32, tag="sc", name=f"sc{h % 2}")
                            scs[h] = sc
                            for u in range(2):
                                ki = 2 * kg + u
                                uu = ki - 4 * nj
                                c0 = 128 * uu if uu > 0 else 0
                                nc.tensor.matmul(
                                    sc[:, u * 512 + c0:(u + 1) * 512],
                                    lhsT=kT[:, h, ki * 128:(ki + 1) * 128],
                                    rhs=qT[:, h, nj * 512 + c0:(nj + 1) * 512],
                                    start=True, stop=True)
                        for h in pair_heads:
                            pt = att_work.tile([128, 1024], bf16, tag="pt", bufs=4,
                                               name=f"pt{h % 2}")
                            pts[h] = pt
                            # columns below 128*uu of a diagonal block are
                            # fully masked; skipped in scores and the P@V matmul
                            nc.scalar.activation(pt[:, g0:1024], scs[h][:, g0:1024], AF.Exp)
                        for h in pair_heads:
                            pt, av = pts[h], pav[h]
                            for u in range(2):
                                ki = 2 * kg + u
                                uu = ki - 4 * nj  # >=0 on diagonal blocks
                                if uu >= 0:
                                    b0 = u * 512 + uu * 128
                                    nc.gpsimd.affine_select(
                                        out=pt[:, b0:b0 + 128],
                                        in_=pt[:, b0:b0 + 128],
                                        compare_op=mybir.AluOpType.is_ge,
                                        fill=0.0,
                                        base=0,
                                        channel_multiplier=-1,
                                        pattern=[[1, 128]],
                                    )
                                    nc.tensor.matmul(av[:, uu * 128:512],
                                                     lhsT=v4[:, ki, h * 65:(h + 1) * 65],
                                                     rhs=pt[:, u * 512 + uu * 128:(u + 1) * 512],
                                                     start=(ki == 0), stop=(ki == nblk - 1))
                                else:
                                    nc.tensor.matmul(av[:],
                                                     lhsT=v4[:, ki, h * 65:(h + 1) * 65],
                                                     rhs=pt[:, u * 512:(u + 1) * 512],
                                                     start=(ki == 0), stop=(ki == nblk - 1))
                    # normalize this pair: row 64 of av is the softmax
                    # denominator; scale rows 0-63 by 1/denom. The reciprocal
                    # runs on ACT as exp(-ln(x)) — both functions live in the
                    # natural_log_exp_and_others table set with the attention
                    # exp, so there is no table switch and no DMA round-trip
                    # (DVE's [1,512] reciprocal costs 3.3us on one lane).
                    araws, rcs = {}, {}
                    for h in pair_heads:
                        araw = nrm_work.tile([64, 512], bf16, tag="araw", bufs=4)
                        nc.vector.tensor_copy(araw[:], pav[h][0:64, :])
                        araws[h] = araw
                    for h in pair_heads:
                        lnd = nrm_work.tile([1, 512], f32, tag="lnd", bufs=2)
                        nc.scalar.activation(lnd[:], pav[h][64:65, :], AF.Ln)
                        rc0 = nrm_work.tile([1, 512], f32, tag="rc0", bufs=2)
                        nc.scalar.activation(rc0[:], lnd[:], AF.Exp, scale=-1.0)
                        rcs[h] = rc0
                    for h in pair_heads:
                        bcs = nrm_work.tile([64, 512], f32, tag="bcs", bufs=4)
                        nc.gpsimd.partition_broadcast(bcs[:], rcs[h][:])
                        if h % 2:
                            om = nrm_work.tile([64, 512], bf16, tag="om")
                            nc.vector.tensor_mul(om[:], araws[h][:], bcs[:])
                            nc.sync.dma_start(out=avt[64:128, hp, qs], in_=om[:])
                        else:
                            nc.vector.tensor_mul(avt[0:64, hp, qs], araws[h][:], bcs[:])

            def o_window(nj):
                for qi in range(4 * nj, 4 * nj + 4):
                    ot = oout_sb.tile([128, S], bf16, tag="ot")
                    for nch in range(NQ):
                        op = chp.tile([128, 512], f32, tag="chain")
                        for p in range(2):
                            nc.tensor.matmul(op[:],
                                             lhsT=avt[:, p, qi * 128:(qi + 1) * 128],
                                             rhs=wo_sb[:, p, nch * 512:(nch + 1) * 512],
                                             start=(p == 0), stop=(p == 1))
                        nc.vector.tensor_copy(ot[:, nch * 512:(nch + 1) * 512], op[:])
                    nc.sync.dma_start(out=out[qi * 128:(qi + 1) * 128, :], in_=ot[:])

            # ---- prologue: window-0 projections. The 8 q/k chains run
            # k-outer across all 8 PSUM banks (borrowing the idle sc/av
            # slots) so the tensor engine keeps pace with the streaming
            # input DMA instead of serializing 2 chains at a time. ----
            pro_tiles = [chp.tile([128, 512], f32, tag="chain", name="pch0"),
                         chp.tile([128, 512], f32, tag="chain", name="pch1"),
                         atp.tile([128, 512], f32, tag="av", name="pav0"),
                         atp.tile([128, 512], f32, tag="av", name="pav1"),
                         atp.tile([128, 1024], f32, tag="sc", name="psc0"),
                         atp.tile([128, 1024], f32, tag="sc", name="psc1")]
            pro_chains = []
            for h in range(NHC):
                for w_sb, dst in ((wq_sb, qT), (wk_sb, kT)):
                    pro_chains.append((h, w_sb, dst))
            pro_aps = [pro_tiles[0][:], pro_tiles[1][:],
                       pro_tiles[2][:], pro_tiles[3][:],
                       pro_tiles[4][:, 0:512], pro_tiles[4][:, 512:1024],
                       pro_tiles[5][:, 0:512], pro_tiles[5][:, 512:1024]]
            for k in range(KT):
                for i, (h, w_sb, dst) in enumerate(pro_chains):
                    nc.tensor.matmul(pro_aps[i],
                                     lhsT=w_sb[:, k, h * 128:(h + 1) * 128],
                                     rhs=ht_all[:, k, 0:512],
                                     start=(k == 0), stop=(k == KT - 1))
            # evacuate prologue chains on ACT (idle during the ramp) so the
            # DVE is free for the V-window casts that gate the first PV
            for i, (h, w_sb, dst) in enumerate(pro_chains):
                nc.scalar.copy(dst[:, h, 0:512], pro_aps[i])
            v_window(0)
            # ---- pipelined main loop ----
            for nj in range(NQ):
                attention(nj)
                if nj + 1 < NQ:
                    qk_window(nj + 1)
                    v_window(nj + 1)
                o_window(nj)
    return nc


def _get_nc():
    global _CACHED_NC
    if _CACHED_NC is None:
        nc = _build_nc()
        if not nc.is_finalized():
            nc.finalize()
        _CACHED_NC = nc
    return _CACHED_NC


def _prep_in_maps(hidden_states, Wq, Wk, Wv, Wo, lambda_param):
    lam = math.tanh(math.log1p(math.exp(float(lambda_param))))
    scale = HD ** -0.5
    in_maps = []
    hTb = [np.ascontiguousarray(hidden_states[b].T).astype(BF) for b in range(2)]
    for core in range(8):
        b, g = divmod(core, 4)
        heads = range(NHC * g, NHC * g + NHC)
        wq_cols, wk_cols = [], []
        for h in heads:
            wq_cols.append(Wq[:, h * 64:(h + 1) * 64] * scale)
            wq_cols.append(Wq[:, (NH + h) * 64:(NH + h + 1) * 64] * (-lam * scale))
            wk_cols.append(Wk[:, h * 64:(h + 1) * 64])
            wk_cols.append(Wk[:, (NH + h) * 64:(NH + h + 1) * 64])
        wv_pad = np.zeros((H, NHC * 65), dtype=np.float32)
        for j, h in enumerate(heads):
            wv_pad[:, j * 65:j * 65 + 64] = Wv[:, h * 64:(h + 1) * 64]
        heads = list(heads)
        wo_sel = np.zeros((128, 2, S), dtype=np.float32)  # head-pair stacked rows
        for p in range(2):
            h0, h1 = heads[2 * p], heads[2 * p + 1]
            wo_sel[0:64, p] = Wo[h0 * 64:(h0 + 1) * 64, :]
            wo_sel[64:128, p] = Wo[h1 * 64:(h1 + 1) * 64, :]
        in_maps.append({
            "hT": hTb[b],
            "wq": np.concatenate(wq_cols, axis=1).astype(BF),
            "wk": np.concatenate(wk_cols, axis=1).astype(BF),
            "wv": wv_pad.astype(BF),
            "wo": np.ascontiguousarray(wo_sel).astype(BF),
        })
    return in_maps


def _mask_is_causal(attention_mask):
    m = np.asarray(attention_mask)
    if m.shape != (2, 1, S, S):
        return False
    neg = np.float32(np.finfo(np.float32).min)
    tri = np.tril(np.ones((S, S), dtype=bool))
    expect = np.where(tri, np.float32(0.0), neg)
    return all(np.array_equal(m[b, 0], expect) for b in range(m.shape[0]))


def _fallback(hidden_states, attention_mask, Wq, Wk, Wv, Wo, lambda_param):
    hs = hidden_states.astype(np.float32)
    lam = math.tanh(math.log1p(math.exp(float(lambda_param))))
    scaling = HD ** -0.5
    B = hs.shape[0]
    out = np.empty((B, S, H), dtype=np.float32)
    for b in range(B):
        q_all = (hs[b] @ Wq).reshape(S, 2 * NH, HD).transpose(1, 0, 2)
        k_all = (hs[b] @ Wk).reshape(S, 2 * NH, HD).transpose(1, 0, 2)
        v = (hs[b] @ Wv).reshape(S, NH, HD).transpose(1, 0, 2)
        acc = np.zeros((S, H), dtype=np.float32)
        for h in range(NH):
            s1 = q_all[h] @ k_all[h].T
            s2 = q_all[NH + h] @ k_all[NH + h].T
            sc = (s1 - lam * s2) * scaling + attention_mask[b, 0]
            sc -= sc.max(axis=-1, keepdims=True)
            p = np.exp(sc)
            p /= p.sum(axis=-1, keepdims=True)
            acc += (p @ v[h]) @ Wo[h * 64:(h + 1) * 64]
        out[b] = acc
    return out


def _run(inputs, trace=False):
    from concourse.bass_utils import run_bass_kernel_spmd

    hidden_states = np.asarray(inputs["hidden_states"], dtype=np.float32)
    attention_mask = np.asarray(inputs["attention_mask"], dtype=np.float32)
    Wq = np.asarray(inputs["Wq"], dtype=np.float32)
    Wk = np.asarray(inputs["Wk"], dtype=np.float32)
    Wv = np.asarray(inputs["Wv"], dtype=np.float32)
    Wo = np.asarray(inputs["Wo"], dtype=np.float32)
    lam_p = inputs["lambda_param"]

    if not _mask_is_causal(attention_mask):
        return _fallback(hidden_states, attention_mask, Wq, Wk, Wv, Wo, lam_p), None

    in_maps = _prep_in_maps(hidden_states, Wq, Wk, Wv, Wo, lam_p)
    nc = _get_nc()
    res = run_bass_kernel_spmd(nc, in_maps, list(range(8)), trace=trace)
    out = np.empty((2, S, H), dtype=np.float32)
    for b in range(2):
        acc = res.results[4 * b]["out"].astype(np.float32)
        for g in range(1, 4):
            acc = acc + res.results[4 * b + g]["out"].astype(np.float32)
        out[b] = acc
    return out, res


def kernel(**inputs):
    out, _ = _run(inputs, trace=False)
    return out


# revision 10
# speedup vs baseline: 1.3197x; 1.3197x over previous
"""Differential attention kernel for 8 Trainium2 NeuronCores.

Sharding: batch x head-group. Core c handles batch b = c//4, heads
[4g, 4g+4) with g = c%4. Each core computes Q/K/V projections for its
heads over the full sequence, causal differential attention, and its
partial O-projection; the host sums the 4 partials per batch.

Differential attention trick: score = (q1.k1 - lam*q2.k2) * scale is a
single K=128 matmul with stacked [q1*scale; -lam*scale*q2] and [k1; k2]
head vectors (scales folded into the projection weights on the host).

Structure: a software pipeline over 4 query windows of 512. The
prologue projects qT/kT/V for window 0; each iteration nj runs
attention for window nj (ACT-bound: exp) while the tensor engine fills
idle cycles with the projections for window nj+1 and the O-projection
+ output DMA of window nj. PSUM budget: scores 2x[128,1024] (4 banks)
+ attention accumulators 2x[65,512] pair-serial (2 banks) + a shared
chain pool for all projection chains (2 banks).

Softmax: scores are computed transposed (keys on partitions, queries
free), exp'd without max subtraction (inputs are bounded; exp is exact
to 2ULP on ACT), and the denominator comes for free from a ones-column
appended to V in the P@V matmul. Causality is applied structurally
(upper blocks skipped, diagonal blocks trimmed to N=512-128*uu and
zeroed post-exp), which the host validates against the attention_mask
input before dispatch.
"""
import math
from contextlib import ExitStack

import numpy as np
import ml_dtypes

S = 2048
H = 2048
NH = 16
HD = 64
NHC = 4          # heads per core
BF = ml_dtypes.bfloat16

_CACHED_NC = None


def _build_nc():
    import concourse.mybir as mybir
    import concourse.tile as tile
    from concourse import bacc

    bf16 = mybir.dt.bfloat16
    f32 = mybir.dt.float32
    AF = mybir.ActivationFunctionType

    nc = bacc.Bacc(None, target_bir_lowering=False)
    hT = nc.declare_dram_parameter("hT", [H, S], bf16, isOutput=False)
    wq = nc.declare_dram_parameter("wq", [H, NHC * 128], bf16, isOutput=False)
    wk = nc.declare_dram_parameter("wk", [H, NHC * 128], bf16, isOutput=False)
    wv = nc.declare_dram_parameter("wv", [H, NHC * 65], bf16, isOutput=False)
    wo = nc.declare_dram_parameter("wo", [128, 2, S], bf16, isOutput=False)
    out = nc.declare_dram_parameter("out", [S, H], bf16, isOutput=True)

    KT = H // 128    # 16 contraction tiles for projections
    NQ = S // 512    # 4 query windows
    NS = S // 128    # 16 seq tiles

    with tile.TileContext(nc) as tc:
        with ExitStack() as ctx:
            # ---- persistent SBUF ----
            sb = ctx.enter_context(tc.tile_pool(name="sb", bufs=1))
            qk_sb = ctx.enter_context(tc.tile_pool(name="qk", bufs=1))
            ht_all = sb.tile([128, KT, S], bf16)          # hidden^T
            wq_sb = sb.tile([128, KT, NHC * 128], bf16)
            wk_sb = sb.tile([128, KT, NHC * 128], bf16)
            wv_sb = sb.tile([128, KT, NHC * 65], bf16)
            wo_sb = sb.tile([128, 2, S], bf16)            # head-pair stacked Wo rows
            qT = qk_sb.tile([128, NHC, S], bf16)          # [q1*s; -lam*s*q2] per head
            kT = qk_sb.tile([128, NHC, S], bf16)          # [k1; k2] per head
            v4 = qk_sb.tile([128, NS, NHC * 65], bf16)    # V tiles + ones cols
            avt = qk_sb.tile([128, 2, S], bf16)           # attn_out^T, head pairs stacked
            # warm the ACT exp table while DMAs stream in
            warm = sb.tile([1, 16], f32)
            nc.vector.memset(warm[:], 0.0)
            nc.scalar.activation(warm[:], warm[:], AF.Exp)

            # ---- input DMAs: the prologue only needs window-0 columns of
            # hT plus wq/wk, so those stream first, k-ordered, spread over
            # three DMA queues so the issue rate keeps up with the tensor
            # engine. The rest of hT / wv / wo follow. ----
            for k in range(KT):
                rows = slice(k * 128, (k + 1) * 128)
                nc.sync.dma_start(out=ht_all[:, k, 0:512], in_=hT[rows, 0:512])
                nc.scalar.dma_start(out=wq_sb[:, k, :], in_=wq[rows, :])
                nc.gpsimd.dma_start(out=wk_sb[:, k, :], in_=wk[rows, :])
            for k in range(KT):
                rows = slice(k * 128, (k + 1) * 128)
                nc.scalar.dma_start(out=wv_sb[:, k, :], in_=wv[rows, :])
                nc.sync.dma_start(out=ht_all[:, k, 512:S], in_=hT[rows, 512:S])
            nc.sync.dma_start(out=wo_sb[:], in_=wo[:, :, :])

            # shared PSUM chain pool: qk/v/o projection chains (2 banks)
            chp = ctx.enter_context(tc.tile_pool(name="chp", bufs=2, space="PSUM"))
            att_work = ctx.enter_context(tc.tile_pool(name="attw", bufs=3))
            nrm_work = ctx.enter_context(tc.tile_pool(name="nrmw", bufs=2))
            oout_sb = ctx.enter_context(tc.tile_pool(name="oout", bufs=3))
            atp = ctx.enter_context(tc.tile_pool(name="atp", bufs=2, space="PSUM"))

            def qk_window(w):
                ws = slice(w * 512, (w + 1) * 512)
                for h in range(NHC):
                    hs = slice(h * 128, (h + 1) * 128)
                    for w_sb, dst in ((wq_sb, qT), (wk_sb, kT)):
                        pp = chp.tile([128, 512], f32, tag="chain")
                        for k in range(KT):
                            nc.tensor.matmul(pp[:],
                                             lhsT=w_sb[:, k, hs],
                                             rhs=ht_all[:, k, ws],
                                             start=(k == 0), stop=(k == KT - 1))
                        nc.vector.tensor_copy(dst[:, h, ws], pp[:])

            def v_window(w):
                for st in range(4 * w, 4 * w + 4):
                    vp = chp.tile([128, 512], f32, tag="chain")
                    for k in range(KT):
                        nc.tensor.matmul(vp[:, 0:NHC * 65],
                                         lhsT=ht_all[:, k, st * 128:(st + 1) * 128],
                                         rhs=wv_sb[:, k, :],
                                         start=(k == 0), stop=(k == KT - 1))
                    nc.vector.tensor_copy(v4[:, st, :], vp[:, 0:NHC * 65])
                    for j in range(NHC):
                        nc.gpsimd.memset(v4[:, st, j * 65 + 64:j * 65 + 65], 1.0)

            def attention(nj):
                qs = slice(nj * 512, (nj + 1) * 512)
                nblk = 4 * nj + 4
                for hp in range(2):
                    pair_heads = (2 * hp, 2 * hp + 1)
                    pav = {}
                    for h in pair_heads:
                        pav[h] = atp.tile([65, 512], f32, tag="av", bufs=2,
                                          name=f"av{h % 2}")
                    # round-robin the two heads per key-block group so one
                    # head's score matmuls hide the other head's exp
                    for kg in range(nblk // 2):
                        scs, pts = {}, {}
                        g0 = 256 if kg == 2 * nj + 1 else 0
                        for h in pair_heads:
                            sc = atp.tile([128, 1024], f32, tag="sc", name=f"sc{h % 2}")
                            scs[h] = sc
                            for u in range(2):
                                ki = 2 * kg + u
                                uu = ki - 4 * nj
                                c0 = 128 * uu if uu > 0 else 0
                                nc.tensor.matmul(
                                    sc[:, u * 512 + c0:(u + 1) * 512],
                                    lhsT=kT[:, h, ki * 128:(ki + 1) * 128],
                                    rhs=qT[:, h, nj * 512 + c0:(nj + 1) * 512],
                                    start=True, stop=True)
                        for h in pair_heads:
                            pt = att_work.tile([128, 1024], bf16, tag="pt", bufs=4,
                                               name=f"pt{h % 2}")
                            pts[h] = pt
                            # columns below 128*uu of a diagonal block are
                            # fully masked; skipped in scores and the P@V matmul
                            nc.scalar.activation(pt[:, g0:1024], scs[h][:, g0:1024], AF.Exp)
                        for h in pair_heads:
                            pt, av = pts[h], pav[h]
                            for u in range(2):
                                ki = 2 * kg + u
                                uu = ki - 4 * nj  # >=0 on diagonal blocks
                                if uu >= 0:
                                    b0 = u * 512 + uu * 128
                                    nc.gpsimd.affine_select(
                                        out=pt[:, b0:b0 + 128],
                                        in_=pt[:, b0:b0 + 128],
                                        compare_op=mybir.AluOpType.is_ge,
                                        fill=0.0,
                                        base=0,
                                        channel_multiplier=-1,
                                        pattern=[[1, 128]],
                                    )
                                    nc.tensor.matmul(av[:, uu * 128:512],
                                                     lhsT=v4[:, ki, h * 65:(h + 1) * 65],
                                                     rhs=pt[:, u * 512 + uu * 128:(u + 1) * 512],
                                                     start=(ki == 0), stop=(ki == nblk - 1))
                                else:
                                    nc.tensor.matmul(av[:],
                                                     lhsT=v4[:, ki, h * 65:(h + 1) * 65],
                                                     rhs=pt[:, u * 512:(u + 1) * 512],
                                                     start=(ki == 0), stop=(ki == nblk - 1))
                    # normalize this pair: row 64 of av is the softmax
                    # denominator; scale rows 0-63 by 1/denom. Evacuate the
                    # PSUM reads first (releases av slots for the next pair),
                    # then run reciprocal on a [128, n] DMA-folded layout so
                    # all DVE lanes work instead of one. Norm DMAs ride the
                    # gpsimd queue to dodge the input/output traffic on sync.
                    # The last window is normalized in two query halves so
                    # the O-projection tail pipelines with the chain.
                    ranges = ((0, 256), (256, 512)) if nj == NQ - 1 else ((0, 512),)
                    dens, araws = {}, {}
                    for h in pair_heads:
                        den = nrm_work.tile([65, 512], f32, tag="den", bufs=4)
                        nc.vector.tensor_copy(den[64:65, :], pav[h][64:65, :])
                        dens[h] = den
                        araw = nrm_work.tile([64, 512], bf16, tag="araw", bufs=4)
                        nc.vector.tensor_copy(araw[:], pav[h][0:64, :])
                        araws[h] = araw
                    for a, b in ranges:
                        nf = (b - a) // 128
                        dfold = nrm_work.tile([128, 8], f32, tag="dfold")
                        for j, h in enumerate(pair_heads):
                            nc.gpsimd.dma_start(out=dfold[:, nf * j:nf * j + nf],
                                                in_=dens[h][64:65, a:b])
                        nc.vector.reciprocal(dfold[:, 0:2 * nf], dfold[:, 0:2 * nf])
                        for j, h in enumerate(pair_heads):
                            rc0 = nrm_work.tile([1, 512], f32, tag="rc0", bufs=2)
                            nc.gpsimd.dma_start(out=rc0[:, a:b],
                                                in_=dfold[:, nf * j:nf * j + nf])
                            bcs = nrm_work.tile([64, 512], f32, tag="bcs", bufs=4)
                            nc.gpsimd.partition_broadcast(bcs[:, a:b], rc0[:, a:b])
                            if h % 2:
                                om = nrm_work.tile([64, 512], bf16, tag="om", bufs=2)
                                nc.vector.tensor_mul(om[:, a:b], araws[h][:, a:b],
                                                     bcs[:, a:b])
                                nc.gpsimd.dma_start(
                                    out=avt[64:128, hp, nj * 512 + a:nj * 512 + b],
                                    in_=om[:, a:b])
                            else:
                                nc.vector.tensor_mul(
                                    avt[0:64, hp, nj * 512 + a:nj * 512 + b],
                                    araws[h][:, a:b], bcs[:, a:b])

            def o_window(nj):
                for qi in range(4 * nj, 4 * nj + 4):
                    ot = oout_sb.tile([128, S], bf16, tag="ot")
                    for nch in range(NQ):
                        op = chp.tile([128, 512], f32, tag="chain")
                        for p in range(2):
                            nc.tensor.matmul(op[:],
                                             lhsT=avt[:, p, qi * 128:(qi + 1) * 128],
                                             rhs=wo_sb[:, p, nch * 512:(nch + 1) * 512],
                                             start=(p == 0), stop=(p == 1))
                        nc.vector.tensor_copy(ot[:, nch * 512:(nch + 1) * 512], op[:])
                    nc.sync.dma_start(out=out[qi * 128:(qi + 1) * 128, :], in_=ot[:])

            # ---- prologue: window-0 projections. The 8 q/k chains run
            # k-outer across all 8 PSUM banks (borrowing the idle sc/av
            # slots) so the tensor engine keeps pace with the streaming
            # input DMA instead of serializing 2 chains at a time. ----
            pro_tiles = [chp.tile([128, 512], f32, tag="chain", name="pch0"),
                         chp.tile([128, 512], f32, tag="chain", name="pch1"),
                         atp.tile([128, 512], f32, tag="av", name="pav0"),
                         atp.tile([128, 512], f32, tag="av", name="pav1"),
                         atp.tile([128, 1024], f32, tag="sc", name="psc0"),
                         atp.tile([128, 1024], f32, tag="sc", name="psc1")]
            pro_chains = []
            for h in range(NHC):
                for w_sb, dst in ((wq_sb, qT), (wk_sb, kT)):
                    pro_chains.append((h, w_sb, dst))
            pro_aps = [pro_tiles[0][:], pro_tiles[1][:],
                       pro_tiles[2][:], pro_tiles[3][:],
                       pro_tiles[4][:, 0:512], pro_tiles[4][:, 512:1024],
                       pro_tiles[5][:, 0:512], pro_tiles[5][:, 512:1024]]
            for k in range(KT):
                for i, (h, w_sb, dst) in enumerate(pro_chains):
                    nc.tensor.matmul(pro_aps[i],
                                     lhsT=w_sb[:, k, h * 128:(h + 1) * 128],
                                     rhs=ht_all[:, k, 0:512],
                                     start=(k == 0), stop=(k == KT - 1))
            # evacuate prologue chains on ACT (idle during the ramp) so the
            # DVE is free for the V-window casts that gate the first PV
            for i, (h, w_sb, dst) in enumerate(pro_chains):
                nc.scalar.copy(dst[:, h, 0:512], pro_aps[i])
            v_window(0)
            # ---- pipelined main loop ----
            for nj in range(NQ):
                attention(nj)
                if nj + 1 < NQ:
                    qk_window(nj + 1)
                    v_window(nj + 1)
                o_window(nj)
    return nc


def _get_nc():
    global _CACHED_NC
    if _CACHED_NC is None:
        nc = _build_nc()
        if not nc.is_finalized():
            nc.finalize()
        _CACHED_NC = nc
    return _CACHED_NC


def _prep_in_maps(hidden_states, Wq, Wk, Wv, Wo, lambda_param):
    lam = math.tanh(math.log1p(math.exp(float(lambda_param))))
    scale = HD ** -0.5
    in_maps = []
    hTb = [np.ascontiguousarray(hidden_states[b].T).astype(BF) for b in range(2)]
    for core in range(8):
        b, g = divmod(core, 4)
        heads = range(NHC * g, NHC * g + NHC)
        wq_cols, wk_cols = [], []
        for h in heads:
            wq_cols.append(Wq[:, h * 64:(h + 1) * 64] * scale)
            wq_cols.append(Wq[:, (NH + h) * 64:(NH + h + 1) * 64] * (-lam * scale))
            wk_cols.append(Wk[:, h * 64:(h + 1) * 64])
            wk_cols.append(Wk[:, (NH + h) * 64:(NH + h + 1) * 64])
        wv_pad = np.zeros((H, NHC * 65), dtype=np.float32)
        for j, h in enumerate(heads):
            wv_pad[:, j * 65:j * 65 + 64] = Wv[:, h * 64:(h + 1) * 64]
        heads = list(heads)
        wo_sel = np.zeros((128, 2, S), dtype=np.float32)  # head-pair stacked rows
        for p in range(2):
            h0, h1 = heads[2 * p], heads[2 * p + 1]
            wo_sel[0:64, p] = Wo[h0 * 64:(h0 + 1) * 64, :]
            wo_sel[64:128, p] = Wo[h1 * 64:(h1 + 1) * 64, :]
        in_maps.append({
            "hT": hTb[b],
            "wq": np.concatenate(wq_cols, axis=1).astype(BF),
            "wk": np.concatenate(wk_cols, axis=1).astype(BF),
            "wv": wv_pad.astype(BF),
            "wo": np.ascontiguousarray(wo_sel).astype(BF),
        })
    return in_maps


def _mask_is_causal(attention_mask):
    m = np.asarray(attention_mask)
    if m.shape != (2, 1, S, S):
        return False
    neg = np.float32(np.finfo(np.float32).min)
    tri = np.tril(np.ones((S, S), dtype=bool))
    expect = np.where(tri, np.float32(0.0), neg)
    return all(np.array_equal(m[b, 0], expect) for b in range(m.shape[0]))


def _fallback(hidden_states, attention_mask, Wq, Wk, Wv, Wo, lambda_param):
    hs = hidden_states.astype(np.float32)
    lam = math.tanh(math.log1p(math.exp(float(lambda_param))))
    scaling = HD ** -0.5
    B = hs.shape[0]
    out = np.empty((B, S, H), dtype=np.float32)
    for b in range(B):
        q_all = (hs[b] @ Wq).reshape(S, 2 * NH, HD).transpose(1, 0, 2)
        k_all = (hs[b] @ Wk).reshape(S, 2 * NH, HD).transpose(1, 0, 2)
        v = (hs[b] @ Wv).reshape(S, NH, HD).transpose(1, 0, 2)
        acc = np.zeros((S, H), dtype=np.float32)
        for h in range(NH):
            s1 = q_all[h] @ k_all[h].T
            s2 = q_all[NH + h] @ k_all[NH + h].T
            sc = (s1 - lam * s2) * scaling + attention_mask[b, 0]
            sc -= sc.max(axis=-1, keepdims=True)
            p = np.exp(sc)
            p /= p.sum(axis=-1, keepdims=True)
            acc += (p @ v[h]) @ Wo[h * 64:(h + 1) * 64]
        out[b] = acc
    return out


def _run(inputs, trace=False):
    from concourse.bass_utils import run_bass_kernel_spmd

    hidden_states = np.asarray(inputs["hidden_states"], dtype=np.float32)
    attention_mask = np.asarray(inputs["attention_mask"], dtype=np.float32)
    Wq = np.asarray(inputs["Wq"], dtype=np.float32)
    Wk = np.asarray(inputs["Wk"], dtype=np.float32)
    Wv = np.asarray(inputs["Wv"], dtype=np.float32)
    Wo = np.asarray(inputs["Wo"], dtype=np.float32)
    lam_p = inputs["lambda_param"]

    if not _mask_is_causal(attention_mask):
        return _fallback(hidden_states, attention_mask, Wq, Wk, Wv, Wo, lam_p), None

    in_maps = _prep_in_maps(hidden_states, Wq, Wk, Wv, Wo, lam_p)
    nc = _get_nc()
    res = run_bass_kernel_spmd(nc, in_maps, list(range(8)), trace=trace)
    out = np.empty((2, S, H), dtype=np.float32)
    for b in range(2):
        acc = res.results[4 * b]["out"].astype(np.float32)
        for g in range(1, 4):
            acc = acc + res.results[4 * b + g]["out"].astype(np.float32)
        out[b] = acc
    return out, res


def kernel(**inputs):
    out, _ = _run(inputs, trace=False)
    return out
